# revision 9
# baseline (speedup 1.0000x reference)
"""Trainium2 Bass kernel for nn_ConditionalMomentEncoder.

Self-contained: takes full unsharded inputs, shards batch-parallel over 8
NeuronCores, runs one Bass/Tile program per core, gathers [32, 512] output.

Math notes (vs the jax reference):
- The ragged argsort/gather is eliminated: invalid slots have fmp == 0, so
  their key rows and logits are exactly 0. Attention over all N=128 slots
  with denominator correction  den = rowsum(exp(logits)) - (N - L)
  reproduces the reference's softmax over its L=96 gathered slots exactly.
- Softmax max-subtraction is skipped: logits are bounded, fp32 exp is safe.
- conv1 is fused with the attention-weighted sum: conv1 is linear over its
  input channels, and the input image channels are O = A.K (A = [224, 128]
  attention matrix incl. fmp/eig value scaling and 1/den, K = keys rows as
  64x64 images). So per batch we transpose A on the PE, contract it with the
  conv1 weights per tap (w'T[n, tap, oc] = sum_ic A[ic, n] w1[oc, ic, tap]),
  and run conv1 directly on the 128 keys images. This removes the O matmul
  ([224, 4096] x 4 per core) and its PSUM evacuation entirely, and cuts
  conv1's PE work by ~2.6x (contraction 128 instead of 224 x interleave).
"""

import sys

sys.path.insert(0, "/opt/trn_rl_repo")

import ml_dtypes
import numpy as np

import concourse.bacc as bacc
import concourse.mybir as mybir
import concourse.tile as tile
from concourse import bass_utils

F32 = mybir.dt.float32
F32R = mybir.dt.float32r
BF16 = mybir.dt.bfloat16
AF = mybir.ActivationFunctionType

B, N, D = 32, 128, 4096
L = 96
IMG = 64
NCORES = 8
BL = B // NCORES  # 4 batches per core
SCALE = float(D) ** -0.5
M_ALL = L + N  # 224 query rows total
M_PAD = 224

_CACHE = {}
LAST_RESULT = None


def _build_program():
    nc = bacc.Bacc("TRN2", target_bir_lowering=False, debug=False)

    keys_d = nc.dram_tensor("keys", [BL, N, D], BF16, kind="ExternalInput")
    keysT_d = nc.dram_tensor("keysT", [BL, N, D], BF16, kind="ExternalInput")
    scales_d = nc.dram_tensor("scales", [BL, N, 4], F32, kind="ExternalInput")
    qt_d = nc.dram_tensor("qt", [128, 32 * M_PAD], BF16, kind="ExternalInput")
    w1f_d = nc.dram_tensor("w1f", [96, 9 * 256], BF16, kind="ExternalInput")
    w1e_d = nc.dram_tensor("w1e", [128, 9 * 256], BF16, kind="ExternalInput")
    w2a_d = nc.dram_tensor("w2a", [128, 9 * 128], BF16, kind="ExternalInput")
    w2b_d = nc.dram_tensor("w2b", [128, 9 * 128], BF16, kind="ExternalInput")
    w3_d = nc.dram_tensor("w3", [128, 9 * 128], BF16, kind="ExternalInput")
    w4_d = nc.dram_tensor("w4", [128, 9 * 64], BF16, kind="ExternalInput")
    lw_d = nc.dram_tensor("lw", [128, 8 * 512], BF16, kind="ExternalInput")
    bias_d = nc.dram_tensor("biasp", [128, 5], F32, kind="ExternalInput")
    linb_d = nc.dram_tensor("linb", [2, 512], BF16, kind="ExternalInput")
    onesc_d = nc.dram_tensor("onesc", [128, 2], F32R, kind="ExternalInput")
    onesr_d = nc.dram_tensor("onesr", [2, 4], BF16, kind="ExternalInput")
    ident_d = nc.dram_tensor("ident", [128, 128], BF16, kind="ExternalInput")
    out_d = nc.dram_tensor("out", [BL, 512], F32, kind="ExternalOutput")

    with tile.TileContext(nc) as tc:
        with (
            tc.tile_pool(name="const", bufs=1) as cpool,
            tc.tile_pool(name="imgs", bufs=1) as ipool,
            tc.tile_pool(name="keys", bufs=3) as kpool,
            tc.tile_pool(name="keysT", bufs=3) as ktpool,
            tc.tile_pool(name="small", bufs=2) as spool,
            tc.tile_pool(name="wpt", bufs=2) as wpool,
            tc.tile_pool(name="gt_ps", bufs=1, space="PSUM") as gt_psum,
            tc.tile_pool(name="tr_ps", bufs=1, space="PSUM") as tr_psum,
            tc.tile_pool(name="dn_ps", bufs=1, space="PSUM") as dn_psum,
            tc.tile_pool(name="wp_ps", bufs=3, space="PSUM") as wp_psum,
            tc.tile_pool(name="cv_ps", bufs=2, space="PSUM") as cv_psum,
            tc.tile_pool(name="dram", bufs=1, space="DRAM") as dpool,
        ):
            # ---- batch-0 critical path DMAs first (PE starts ASAP) ----
            # interleave keysT0/qt chunks so GT(0) matmuls stream as data lands
            keysT0 = ktpool.tile([128, D], BF16, tag="kt")
            qt_sb = cpool.tile([128, 32 * M_PAD], BF16)
            for ch in range(4):
                nc.sync.dma_start(keysT0[:, ch * 1024:(ch + 1) * 1024],
                                  keysT_d[0][:, ch * 1024:(ch + 1) * 1024])
                nc.sync.dma_start(qt_sb[:, ch * M_PAD * 8:(ch + 1) * M_PAD * 8],
                                  qt_d[:, ch * M_PAD * 8:(ch + 1) * M_PAD * 8])
            sc0 = spool.tile([128, 4], F32, tag="sc")
            nc.sync.dma_start(sc0[:], scales_d[0])
            onesc = cpool.tile([128, 2], F32R)
            nc.sync.dma_start(onesc[:], onesc_d[:])
            ident = cpool.tile([128, 128], BF16)
            nc.sync.dma_start(ident[:], ident_d[:])

            # ---- PE warm-up + act-table preload while head DMAs stream ----
            # (TRN2 PE clock ramps with continuous execution; the scalar
            # engine's first Exp triggers a 1.3us act-table load)
            wtile = cpool.tile([128, 224], BF16)
            nc.vector.memset(wtile[:], 0.0078125)
            wact = cpool.tile([128, 2], F32)
            nc.scalar.activation(wact[:, 0:1], wtile[:, 0:1], AF.Exp,
                                 bias=0.0, scale=1.0)
            for _ in range(18):
                wps = cv_psum.tile([128, 224], F32, tag="cv")
                nc.tensor.matmul(wps[:], wtile[:, 0:128], wtile[:],
                                 start=True, stop=True)

            # conv1-fusion weights, needed early (w'(0) runs ~t+17us)
            w1f = cpool.tile([96, 9 * 256], BF16)
            nc.sync.dma_start(w1f[:], w1f_d[:])
            w1e = cpool.tile([128, 9 * 256], BF16)
            nc.sync.dma_start(w1e[:], w1e_d[:])

            # keysT1 before keys0: GT(1) runs just before conv1'(0)
            kt1 = ktpool.tile([128, D], BF16, tag="kt")
            nc.sync.dma_start(kt1[:], keysT_d[1])
            sc1 = spool.tile([128, 4], F32, tag="sc")
            nc.sync.dma_start(sc1[:], scales_d[1])

            keys0 = kpool.tile([128, D], BF16, tag="k")
            nc.sync.dma_start(keys0[:], keys_d[0])

            # persistent working buffers (bf16 conv stack)
            c1a = ipool.tile([128, 1024], BF16)
            c1b = ipool.tile([128, 1024], BF16)
            c2o = ipool.tile([128, 256], BF16)
            c3o = ipool.tile([128, 64], BF16)
            f_all = ipool.tile([64, BL * 16], BF16)

            c1a3 = c1a[:].rearrange("p (h w) -> p h w", h=32, w=32)
            c1b3 = c1b[:].rearrange("p (h w) -> p h w", h=32, w=32)
            c2o3 = c2o[:].rearrange("p (h w) -> p h w", h=16, w=16)
            c3o3 = c3o[:].rearrange("p (h w) -> p h w", h=8, w=8)

            # mid-priority weights (conv2-4 of batch 0 runs ~t+27us)
            biasp = cpool.tile([128, 5], F32)
            nc.sync.dma_start(biasp[:], bias_d[:])
            w2a = cpool.tile([128, 9 * 128], BF16)
            nc.sync.dma_start(w2a[:], w2a_d[:])
            w2b = cpool.tile([128, 9 * 128], BF16)
            nc.sync.dma_start(w2b[:], w2b_d[:])
            w3 = cpool.tile([128, 9 * 128], BF16)
            nc.sync.dma_start(w3[:], w3_d[:])
            w4 = cpool.tile([128, 9 * 64], BF16)
            nc.sync.dma_start(w4[:], w4_d[:])

            # prefetch batches 1-2 keys + batch-2 keysT
            bt = {1: (None, kt1, sc1)}
            k1 = kpool.tile([128, D], BF16, tag="k")
            nc.sync.dma_start(k1[:], keys_d[1])
            bt[1] = (k1, kt1, sc1)
            t2 = ktpool.tile([128, D], BF16, tag="kt")
            nc.sync.dma_start(t2[:], keysT_d[2])
            s2 = spool.tile([128, 4], F32, tag="sc")
            nc.sync.dma_start(s2[:], scales_d[2])
            k2 = kpool.tile([128, D], BF16, tag="k")
            nc.sync.dma_start(k2[:], keys_d[2])
            bt[2] = (k2, t2, s2)

            # late loads (linear runs last)
            lw = cpool.tile([128, 8 * 512], BF16)
            nc.sync.dma_start(lw[:], lw_d[:])
            linb = cpool.tile([2, 512], BF16)
            nc.sync.dma_start(linb[:], linb_d[:])
            ones_row = cpool.tile([2, 4], BF16)
            nc.sync.dma_start(ones_row[:], onesr_d[:])

            fr = cpool.tile([128, 4 * 8], BF16)  # col = 8*b + j

            def rtap(ky, kx, S):
                # restricted tap for unpadded stride-2 conv, output side S
                oy0 = 1 if ky == 0 else 0
                ox0 = 1 if kx == 0 else 0
                return (oy0, S - oy0, ox0, S - ox0,
                        2 * oy0 + ky - 1, 2 * ox0 + kx - 1)
            TAPS1 = [(1, 1), (0, 0), (0, 1), (0, 2), (1, 0), (1, 2),
                     (2, 0), (2, 1), (2, 2)]

            def load_batch(b):
                if b == 0:
                    return keys0, keysT0, sc0
                if b in bt:
                    return bt[b]
                keysT_sb = ktpool.tile([128, D], BF16, tag="kt")
                nc.sync.dma_start(keysT_sb[:], keysT_d[b])
                keys_sb = kpool.tile([128, D], BF16, tag="k")
                nc.sync.dma_start(keys_sb[:], keys_d[b])
                sc = spool.tile([128, 4], F32, tag="sc")
                nc.sync.dma_start(sc[:], scales_d[b])
                return keys_sb, keysT_sb, sc

            def gt_phase(b, keysT_sb, sc):
                """G^T matmul + exp + et2; returns (et, et2)."""
                gt_ps = gt_psum.tile([128, M_PAD], F32, tag="gt")
                for c in range(32):
                    nc.tensor.matmul(
                        gt_ps[:],
                        keysT_sb[:, c * 128:(c + 1) * 128],
                        qt_sb[:, c * M_PAD:(c + 1) * M_PAD],
                        start=(c == 0),
                        stop=(c == 31),
                    )
                # E^T = exp(G^T * scale_col); et2 = E^T * val_col (bf16)
                et = spool.tile([128, M_ALL], F32R, tag="et")
                nc.scalar.activation(et[:, 0:96], gt_ps[:, 0:96], AF.Exp,
                                     bias=0.0, scale=sc[:, 0:1])
                nc.scalar.activation(et[:, 96:224], gt_ps[:, 96:224], AF.Exp,
                                     bias=0.0, scale=sc[:, 1:2])
                et2 = spool.tile([128, M_ALL], BF16, tag="et2")
                nc.vector.tensor_scalar_mul(et2[:, 0:96], et[:, 0:96], sc[:, 2:3])
                nc.vector.tensor_scalar_mul(et2[:, 96:224], et[:, 96:224], sc[:, 3:4])
                return et, et2

            def dt_phase(b, et, et2):
                """den matmuls + transpose of et2 (PE); rden + A evacs
                (vector) trail into the surrounding conv stack."""
                dn_ps = dn_psum.tile([128, 4], F32, tag="dn")
                nc.tensor.matmul(dn_ps[0:96, 0:2], et[:, 0:96], onesc[:],
                                 start=True, stop=True)
                nc.tensor.matmul(dn_ps[0:128, 2:4], et[:, 96:224], onesc[:],
                                 start=True, stop=True)
                # transpose et2 -> [m, n] (bf16 psum)
                tr_ps = tr_psum.tile([128, 256], BF16, tag="tr")
                nc.tensor.matmul(tr_ps[0:96, 0:128], et2[:, 0:96], ident[:],
                                 start=True, stop=True, is_transpose=True)
                nc.tensor.matmul(tr_ps[0:128, 128:256], et2[:, 96:224], ident[:],
                                 start=True, stop=True, is_transpose=True)

                rden = spool.tile([128, 3], F32, tag="rden")
                nc.vector.tensor_scalar_add(rden[0:96, 0:1], dn_ps[0:96, 0:1], -32.0)
                nc.vector.reciprocal(rden[0:96, 1:2], rden[0:96, 0:1])
                nc.vector.reciprocal(rden[0:128, 2:3], dn_ps[0:128, 2:3])

                a_f = spool.tile([96, 128], BF16, tag="af")
                nc.vector.tensor_scalar_mul(a_f[:], tr_ps[0:96, 0:128],
                                            rden[0:96, 1:2])
                a_e = spool.tile([128, 128], BF16, tag="ae")
                nc.vector.tensor_scalar_mul(a_e[:], tr_ps[0:128, 128:256],
                                            rden[0:128, 2:3])
                return a_f, a_e

            def w_phase(b, a_f, a_e):
                """w'T[n, tap, oc] = sum_ic A[ic, n] w1[oc, ic, tap]"""
                wpT = wpool.tile([128, 9 * 256], BF16, tag="wp")
                for t in range(9):
                    wp_ps = wp_psum.tile([128, 256], F32, tag="wp")
                    nc.tensor.matmul(wp_ps[:], a_f[:],
                                     w1f[:, t * 256:(t + 1) * 256],
                                     start=True, stop=False)
                    nc.tensor.matmul(wp_ps[:], a_e[:],
                                     w1e[:, t * 256:(t + 1) * 256],
                                     start=False, stop=True)
                    dst = wpT[:, t * 256:(t + 1) * 256]
                    if t % 2 == 0:
                        nc.vector.tensor_copy(dst, wp_ps[:])
                    else:
                        nc.scalar.activation(dst, wp_ps[:], AF.Copy,
                                             bias=0.0, scale=1.0)
                return wpT

            def conv1_og(b, keys_sb, wpT, og):
                """conv1' half (one 128-oc group) on keys images."""
                keys3 = keys_sb[:].rearrange("p (h w) -> p h w", h=64, w=64)
                c1dst = c1a if og == 0 else c1b
                for fc in range(2):
                    ps = cv_psum.tile([128, 512], F32, tag="cv")
                    ps3 = ps[:].rearrange("p (a b) -> p a b", a=16, b=32)
                    for ti, (ky, kx) in enumerate(TAPS1):
                        oy0 = 1 if (ky == 0 and fc == 0) else 16 * fc
                        ny = 16 * (fc + 1) - oy0
                        ox0 = 1 if kx == 0 else 0
                        nx = 32 - ox0
                        iy0 = 2 * oy0 + ky - 1
                        ix0 = 2 * ox0 + kx - 1
                        nc.tensor.matmul(
                            ps3[:, oy0 - 16 * fc:oy0 - 16 * fc + ny,
                                ox0:32],
                            wpT[:, (3 * ky + kx) * 256 + og * 128:
                                (3 * ky + kx) * 256 + og * 128 + 128],
                            keys3[:, iy0:iy0 + 2 * ny - 1:2,
                                  ix0:ix0 + 2 * nx - 1:2],
                            start=(ti == 0),
                            stop=(ti == 8),
                        )
                    nc.scalar.activation(
                        c1dst[:, 512 * fc:512 * (fc + 1)], ps[:],
                        AF.Relu, bias=biasp[:, og:og + 1], scale=1.0,
                    )

            def conv_rest(b):
                # ---- conv2: 256 -> 128 ch, 32x32 -> 16x16 ----
                ps = cv_psum.tile([128, 256], F32, tag="cv")
                ps3 = ps[:].rearrange("p (a b) -> p a b", a=16, b=16)
                for ti, (ky, kx) in enumerate(TAPS1):
                    rr = rtap(ky, kx, 16)
                    for ic in range(2):
                        srci = c1a3 if ic == 0 else c1b3
                        wt = w2a if ic == 0 else w2b
                        nc.tensor.matmul(
                            ps3[:, rr[0]:rr[0] + rr[1], rr[2]:rr[2] + rr[3]],
                            wt[:, (3 * ky + kx) * 128:(3 * ky + kx) * 128 + 128],
                            srci[:, rr[4]:rr[4] + 2 * rr[1] - 1:2,
                                 rr[5]:rr[5] + 2 * rr[3] - 1:2],
                            start=(ti == 0 and ic == 0),
                            stop=(ti == 8 and ic == 1),
                        )
                nc.scalar.activation(c2o[:], ps[:], AF.Relu,
                                     bias=biasp[:, 2:3], scale=1.0)

                # ---- conv3: 128 -> 128 ch, 16x16 -> 8x8 ----
                ps = cv_psum.tile([128, 64], F32, tag="cv")
                ps3 = ps[:].rearrange("p (a b) -> p a b", a=8, b=8)
                for ti, (ky, kx) in enumerate(TAPS1):
                    rr = rtap(ky, kx, 8)
                    nc.tensor.matmul(
                        ps3[:, rr[0]:rr[0] + rr[1], rr[2]:rr[2] + rr[3]],
                        w3[:, (3 * ky + kx) * 128:(3 * ky + kx) * 128 + 128],
                        c2o3[:, rr[4]:rr[4] + 2 * rr[1] - 1:2,
                             rr[5]:rr[5] + 2 * rr[3] - 1:2],
                        start=(ti == 0), stop=(ti == 8),
                    )
                nc.scalar.activation(c3o[:], ps[:], AF.Relu,
                                     bias=biasp[:, 3:4], scale=1.0)

                # ---- conv4: 128 -> 64 ch, 8x8 -> 4x4 ----
                ps = cv_psum.tile([64, 16], F32, tag="cv")
                ps3 = ps[:].rearrange("p (a b) -> p a b", a=4, b=4)
                for ti, (ky, kx) in enumerate(TAPS1):
                    rr = rtap(ky, kx, 4)
                    nc.tensor.matmul(
                        ps3[:, rr[0]:rr[0] + rr[1], rr[2]:rr[2] + rr[3]],
                        w4[:, (3 * ky + kx) * 64:(3 * ky + kx) * 64 + 64],
                        c3o3[:, rr[4]:rr[4] + 2 * rr[1] - 1:2,
                             rr[5]:rr[5] + 2 * rr[3] - 1:2],
                        start=(ti == 0), stop=(ti == 8),
                    )
                nc.scalar.activation(f_all[:, b * 16:(b + 1) * 16], ps3[:],
                                     AF.Relu, bias=biasp[0:64, 4:5], scale=1.0)

                # direct SBUF->SBUF reshuffle: fr[p, 8b+j] = flat[b, 8p+j]
                # (element streams of [64,16] and [128,8] coincide)
                nc.sync.dma_start(fr[:, b * 8:(b + 1) * 8],
                                  f_all[:, b * 16:(b + 1) * 16])

            # ---- software-pipelined schedule ----
            # PE order: GT(b) | conv1'og0(b-1) | den/transp(b) |
            #           conv1'og1+conv2-4(b-1) | w'(b) | GT(b+1) ...
            # exp/et2(b) overlap conv1'og0(b-1); rden/A-evacs overlap the
            # rest of the conv stack; wpT(b) evacs overlap GT(b+1).
            keys_sb0, keysT_sb0, sc_0 = load_batch(0)
            et, et2 = gt_phase(0, keysT_sb0, sc_0)
            a_f, a_e = dt_phase(0, et, et2)
            state = (keys_sb0, w_phase(0, a_f, a_e))
            for b in range(1, BL):
                keys_sb, keysT_sb, sc = load_batch(b)
                et, et2 = gt_phase(b, keysT_sb, sc)
                pk, pw = state
                conv1_og(b - 1, pk, pw, 0)
                a_f, a_e = dt_phase(b, et, et2)
                conv1_og(b - 1, pk, pw, 1)
                conv_rest(b - 1)
                state = (keys_sb, w_phase(b, a_f, a_e))
            pk, pw = state
            conv1_og(BL - 1, pk, pw, 0)
            conv1_og(BL - 1, pk, pw, 1)
            conv_rest(BL - 1)

            # ---- linear: out^T[b, o] = sum_f flat[b, f] lin_w[o, f] + lin_b ----
            lin_ps = cv_psum.tile([4, 512], F32, tag="cv")
            for j in range(8):
                nc.tensor.matmul(
                    lin_ps[:],
                    fr[:, j::8],
                    lw[:, 512 * j:512 * j + 512],
                    start=(j == 0), stop=False,
                )
            nc.tensor.matmul(lin_ps[:], ones_row[:], linb[:],
                             start=False, stop=True)
            out_sb = cpool.tile([4, 512], F32)
            nc.vector.tensor_copy(out_sb[:], lin_ps[:])
            nc.sync.dma_start(out_d[:], out_sb[:])

    nc.finalize()
    return nc


def _prep_inputs(inputs):
    keys = np.ascontiguousarray(inputs["keys"], dtype=np.float32)
    fmp = np.asarray(inputs["first_moment_projections"], dtype=np.float32)
    eig = np.asarray(inputs["eigen_values"], dtype=np.float32)
    qf = np.asarray(inputs["queries_fmp"], dtype=np.float32)
    qe = np.asarray(inputs["queries_eig"], dtype=np.float32)
    bf = ml_dtypes.bfloat16

    q_all = np.concatenate([qf, qe], axis=0)  # [224, 4096]
    qt = np.ascontiguousarray(
        q_all.T.reshape(32, 128, M_PAD).transpose(1, 0, 2)
        .reshape(128, 32 * M_PAD).astype(bf))

    def conv_w(w):
        # [oc, ic, 3, 3] -> [ic, ky*3+kx, oc] flattened [ic, 9*oc], bf16
        oc, ic = w.shape[0], w.shape[1]
        return np.ascontiguousarray(
            np.asarray(w, np.float32).transpose(1, 2, 3, 0)
            .reshape(ic, 9 * oc).astype(bf))

    w1t = conv_w(inputs["w1"])
    w2t = conv_w(inputs["w2"])
    w3t = conv_w(inputs["w3"])
    w4t = conv_w(inputs["w4"])

    # lw[p, j*512 + o] = lin_w[o, p*8 + j]  (fr[p, 8b+j] = flat[b, 8p+j])
    lw = np.ascontiguousarray(
        np.asarray(inputs["lin_w"], np.float32).T
        .reshape(128, 8, 512).reshape(128, 8 * 512).astype(bf))

    biasp = np.zeros((128, 5), np.float32)
    b1 = np.asarray(inputs["b1"], np.float32)
    biasp[:, 0] = b1[0:128]
    biasp[:, 1] = b1[128:256]
    biasp[:, 2] = np.asarray(inputs["b2"], np.float32)
    biasp[:, 3] = np.asarray(inputs["b3"], np.float32)
    biasp[0:64, 4] = np.asarray(inputs["b4"], np.float32)
    linb = np.zeros((2, 512), np.float32)
    linb[0] = np.asarray(inputs["lin_b"], np.float32)

    shared = {
        "qt": qt,
        "w1f": np.ascontiguousarray(w1t[0:96]),
        "w1e": np.ascontiguousarray(w1t[96:224]),
        "w2a": np.ascontiguousarray(w2t[0:128]),
        "w2b": np.ascontiguousarray(w2t[128:256]),
        "w3": w3t,
        "w4": w4t,
        "lw": lw,
        "biasp": biasp,
        "linb": linb.astype(bf),
        "onesc": np.ones((128, 2), np.float32),
        "onesr": np.ones((2, 4), bf),
        "ident": np.eye(128, dtype=bf),
    }

    in_maps = []
    for c in range(NCORES):
        sl = slice(c * BL, (c + 1) * BL)
        kc = keys[sl]
        ktc = np.ascontiguousarray(
            kc.transpose(0, 2, 1).reshape(BL, 32, 128, 128)
            .transpose(0, 2, 1, 3).reshape(BL, 128, D).astype(bf))
        scl = np.zeros((BL, N, 4), np.float32)
        scl[:, :, 0] = fmp[sl] * SCALE
        scl[:, :, 1] = eig[sl] * SCALE
        scl[:, :, 2] = fmp[sl]
        scl[:, :, 3] = eig[sl]
        m = {"keys": np.ascontiguousarray(kc.astype(bf)), "keysT": ktc,
             "scales": np.ascontiguousarray(scl)}
        m.update(shared)
        in_maps.append(m)
    return in_maps


def kernel(**inputs):
    global LAST_RESULT
    if "nc" not in _CACHE:
        _CACHE["nc"] = _build_program()
    nc = _CACHE["nc"]
    in_maps = _prep_inputs(inputs)
    res = bass_utils.run_bass_kernel_spmd(nc, in_maps, core_ids=list(range(NCORES)))
    LAST_RESULT = res
    out = np.concatenate([res.results[c]["out"] for c in range(NCORES)], axis=0)
    return out.astype(np.float32)
